# revision 2
# baseline (speedup 1.0000x reference)
"""HSTU positional encoder on Trainium2, SPMD across 8 NeuronCores.

out[t] = seq_embeddings[t] * sqrt(D) + pos_weight[pos[t]]

pos[t] is derived from the ragged sequence structure (seq_offsets /
seq_lengths) on the host (tiny int metadata), then the heavy memory work
(embeddings read, table-row gather, output write: 48MB per core) runs on
device. Tokens are split evenly across the 8 cores (each token's work is
independent once pos[t] is known, so equal-size shards beat whole-sequence
grouping for load balance).
"""

import numpy as np

import concourse.bacc as bacc
import concourse.bass as bass
import concourse.mybir as mybir
import concourse.tile as tile
from concourse.bass_utils import run_bass_kernel_spmd

N_CORES = 8
TOTAL = 65536
D = 512
TABLE_ROWS = 8192
PART = 128
TOK_PER_CORE = TOTAL // N_CORES      # 8192
TILES = TOK_PER_CORE // PART         # 64 token-tiles of 128 tokens
K = 4                                # token-tiles fused per iteration
ITERS = TILES // K                   # 16
ALPHA = float(np.sqrt(D))

_cache: dict = {}


def _build_nc():
    nc = bacc.Bacc("TRN2", target_bir_lowering=False, debug=False)
    emb = nc.dram_tensor("emb", [TOK_PER_CORE, D], mybir.dt.float32,
                         kind="ExternalInput")
    idx = nc.dram_tensor("idx", [PART, TILES], mybir.dt.int32,
                         kind="ExternalInput")
    table = nc.dram_tensor("table", [TABLE_ROWS, D], mybir.dt.float32,
                           kind="ExternalInput")
    out = nc.dram_tensor("out", [TOK_PER_CORE, D], mybir.dt.float32,
                         kind="ExternalOutput")

    # iteration i, SBUF column block k, partition p <-> token (i*K+k)*128+p
    emb_v = emb.ap().rearrange("(n k p) d -> n p k d", k=K, p=PART)
    out_v = out.ap().rearrange("(n k p) d -> n p k d", k=K, p=PART)

    with tile.TileContext(nc) as tc:
        with (
            tc.tile_pool(name="idxp", bufs=1) as idxp,
            tc.tile_pool(name="sbuf", bufs=3) as pool,
        ):
            idx_sb = idxp.tile([PART, TILES], mybir.dt.int32)
            nc.sync.dma_start(idx_sb[:], idx.ap())
            for i in range(ITERS):
                e = pool.tile([PART, K * D], mybir.dt.float32, tag="emb")
                nc.sync.dma_start(
                    e[:].rearrange("p (k d) -> p k d", k=K), emb_v[i])
                g = pool.tile([PART, K * D], mybir.dt.float32, tag="gat")
                nc.gpsimd.indirect_dma_start(
                    out=g[:],
                    out_offset=None,
                    in_=table.ap(),
                    in_offset=bass.IndirectOffsetOnAxis(
                        ap=idx_sb[:, i * K:(i + 1) * K], axis=0),
                )
                o = pool.tile([PART, K * D], mybir.dt.float32, tag="out")
                nc.scalar.mul(o[:], e[:], ALPHA)
                nc.vector.tensor_add(o[:], o[:], g[:])
                nc.sync.dma_start(
                    out_v[i], o[:].rearrange("p (k d) -> p k d", k=K))
    nc.compile()
    return nc


def _get_nc():
    if "nc" not in _cache:
        _cache["nc"] = _build_nc()
    return _cache["nc"]


def _pos_indices(seq_lengths, seq_offsets, total):
    offsets = np.asarray(seq_offsets).astype(np.int64)
    lens = np.asarray(seq_lengths).astype(np.int64)
    tok = np.arange(total, dtype=np.int64)
    seg = np.searchsorted(offsets, tok, side="right") - 1
    high = np.minimum(lens, TABLE_ROWS - 1)
    pos = high[seg] - (tok - offsets[seg])
    return np.clip(pos, 0, TABLE_ROWS - 1).astype(np.int32)


def _run(max_seq_len, seq_lengths, seq_offsets, seq_embeddings, pos_weight,
         trace=False):
    emb = np.ascontiguousarray(np.asarray(seq_embeddings, dtype=np.float32))
    table = np.ascontiguousarray(np.asarray(pos_weight, dtype=np.float32))
    pos = _pos_indices(seq_lengths, seq_offsets, emb.shape[0])
    in_maps = []
    for c in range(N_CORES):
        sl = slice(c * TOK_PER_CORE, (c + 1) * TOK_PER_CORE)
        idx_t = np.ascontiguousarray(pos[sl].reshape(TILES, PART).T)
        in_maps.append({"emb": emb[sl], "idx": idx_t, "table": table})
    res = run_bass_kernel_spmd(_get_nc(), in_maps, list(range(N_CORES)),
                               trace=trace)
    full = np.concatenate([res.results[c]["out"] for c in range(N_CORES)],
                          axis=0)
    return full, res


def kernel(max_seq_len, seq_lengths, seq_offsets, seq_embeddings, pos_weight):
    full, _ = _run(max_seq_len, seq_lengths, seq_offsets, seq_embeddings,
                   pos_weight)
    return full
